# revision 7
# baseline (speedup 1.0000x reference)
"""Trainium2 Bass kernel for BaseGraphAttNet (graph attention, bs=8, N=2048, H=512).

Strategy (data-parallel over batch, one batch per NeuronCore, 8 cores):
  device, per core (batch b):
    phase A: V = feats_b @ fc_w.T                          (PE, bf16)
    phase B: e^T[j,i] = adj_b[i,j] * exp(leaky(q[i]+k[j])) (ACT Prelu+Exp, DVE mask-mult)
    phase C: unnorm_out = e^T.T @ V, denom = ones.T @ e^T  (PE, bf16)
  host:
    transposes (adj^T, feats^T), q/k vectors (tiny rank-1 projections),
    final normalize + residual: out = unnorm_out / denom + fc_b + feats.
    (fc_b moves out of V because softmax rows sum to 1: atten @ (X+b) =
    atten @ X + b.)

Key numerics facts:
  - masked logits for non-edges are ~-1e9, so exp underflows to exactly 0.0 in
    fp32; e = adj * exp(leaky(q_i+k_j)) reproduces the reference row-softmax
    after division by the row sum.
  - q_i errors are common to softmax row i and cancel in the normalization, so
    q may be broadcast through a bf16 K=1 matmul; k stays exact fp32 (ACT bias).
"""

import os
import sys
from contextlib import ExitStack

import numpy as np

sys.path.insert(0, "/opt/trn_rl_repo")

import ml_dtypes

BS, N, H = 8, 2048, 512
NCORES = 8
PART = 128
NT = N // PART  # 16 node tiles (both i and j)
HC = H // PART  # 4 contraction chunks for phase A
NIC = N // H  # 4 i-chunks of 512 for the denominator rows
LEAKY = 0.01
GJ = 4  # j-tiles per adjacency DMA (1 MB fp8 transfers)
GO = 4  # i-tiles per output DMA (1 MB fp32 transfers)

# Prelu(x, alpha) == LeakyReLU and lives in the same ACT table set as Exp
# (exp_and_others), so no table reloads. Fallback (False) uses
# exp(leaky(s)) = max(exp(s), exp(0.01*s)) with two Exp passes + DVE max.
USE_PRELU = True

_PROGRAM_CACHE = {}


def _build_program():
    import concourse.bacc as bacc
    import concourse.mybir as mybir
    import concourse.tile as tile

    f32 = mybir.dt.float32
    bf16 = mybir.dt.bfloat16
    fp8 = mybir.dt.float8e4
    AF = mybir.ActivationFunctionType
    OP = mybir.AluOpType

    nc = bacc.Bacc()

    adjT = nc.declare_dram_parameter("adjT", [N, N], fp8, isOutput=False)
    featsT = nc.declare_dram_parameter("featsT", [H, N], bf16, isOutput=False)
    fcwT = nc.declare_dram_parameter("fcwT", [H, H], bf16, isOutput=False)
    qv = nc.declare_dram_parameter("qv", [1, N], bf16, isOutput=False)
    kv = nc.declare_dram_parameter("kv", [PART, NT], f32, isOutput=False)
    kv2 = nc.declare_dram_parameter("kv2", [PART, NT], f32, isOutput=False)
    out = nc.declare_dram_parameter("out", [N, H], f32, isOutput=True)
    den = nc.declare_dram_parameter("den", [1, N], f32, isOutput=True)

    with tile.TileContext(nc) as tc, ExitStack() as ctx:
        const = ctx.enter_context(tc.tile_pool(name="const", bufs=1))
        vpool = ctx.enter_context(tc.tile_pool(name="vpool", bufs=1))
        epool = ctx.enter_context(tc.tile_pool(name="epool", bufs=1))
        apool = ctx.enter_context(tc.tile_pool(name="apool", bufs=2))
        work = ctx.enter_context(tc.tile_pool(name="work", bufs=3))
        opool = ctx.enter_context(tc.tile_pool(name="opool", bufs=2))
        psA = ctx.enter_context(tc.tile_pool(name="psA", bufs=1, space="PSUM"))
        psD = ctx.enter_context(tc.tile_pool(name="psD", bufs=4, space="PSUM"))
        psC = ctx.enter_context(tc.tile_pool(name="psC", bufs=2, space="PSUM"))

        # ---- small loads first (q broadcast gates the ACT pipeline) ----
        qrow_sb = const.tile([1, N], bf16)
        nc.sync.dma_start(out=qrow_sb, in_=qv[:])
        kc_sb = const.tile([PART, NT], f32)  # k[j] per-partition, j-tile per col
        nc.sync.dma_start(out=kc_sb, in_=kv[:])
        if not USE_PRELU:
            kc2_sb = const.tile([PART, NT], f32)  # 0.01*k[j]
            nc.sync.dma_start(out=kc2_sb, in_=kv2[:])
        ones_row = const.tile([1, PART], bf16)
        nc.vector.memset(ones_row, 1.0)
        ones_col = const.tile([PART, 1], bf16)
        nc.vector.memset(ones_col, 1.0)
        fcwT_sb = const.tile([PART, HC, H], bf16)
        nc.sync.dma_start(
            out=fcwT_sb, in_=fcwT[:].rearrange("(c p) n -> p c n", p=PART)
        )
        featsT_sb = const.tile([PART, HC, N], bf16)
        nc.sync.dma_start(
            out=featsT_sb, in_=featsT[:].rearrange("(c p) i -> p c i", p=PART)
        )

        # q broadcast via K=1 matmul: ones[1,128].T @ q_row[1,512] per chunk
        qb_sb = const.tile([PART, N], f32)
        for ic in range(NIC):
            pq = psA.tile([PART, H], f32, tag="pa")
            nc.tensor.matmul(
                pq,
                lhsT=ones_row,
                rhs=qrow_sb[:, ic * H : (ic + 1) * H],
                start=True,
                stop=True,
            )
            nc.vector.tensor_copy(out=qb_sb[:, ic * H : (ic + 1) * H], in_=pq)

        # ---- phase A: V = feats @ fc_w.T (bias folded to host), bf16 ----
        V_sb = vpool.tile([PART, NT, H], bf16)
        for t in range(NT):
            pa = psA.tile([PART, H], f32, tag="pa")
            for c in range(HC):
                nc.tensor.matmul(
                    pa,
                    lhsT=featsT_sb[:, c, t * PART : (t + 1) * PART],
                    rhs=fcwT_sb[:, c, :],
                    start=(c == 0),
                    stop=(c == HC - 1),
                )
            nc.vector.tensor_copy(out=V_sb[:, t, :], in_=pa)

        # ---- phase B: e^T tiles [j-part, i-free], bf16; denom rows on PE ----
        e_tiles = []
        for j in range(NT):
            e_tiles.append(epool.tile([PART, N], bf16, tag=f"e{j}", name=f"e{j}"))
        pden = []
        for ic in range(NIC):
            pden.append(
                psD.tile([1, H], f32, tag="pden", name=f"pden{ic}")
            )
        for g in range(NT // GJ):
            adj_t = apool.tile([PART, GJ, N], fp8, tag="adj")
            nc.sync.dma_start(
                out=adj_t,
                in_=adjT[:].rearrange("(g c p) i -> g p c i", c=GJ, p=PART)[g],
            )
            for jj in range(GJ):
                j = g * GJ + jj
                if USE_PRELU:
                    t_sb = work.tile([PART, N], f32, tag="t")
                    nc.scalar.activation(
                        out=t_sb,
                        in_=qb_sb,
                        func=AF.Prelu,
                        bias=kc_sb[:, j : j + 1],
                        scale=1.0,
                        alpha=LEAKY,
                    )
                    e0 = work.tile([PART, N], bf16, tag="e0")
                    nc.scalar.activation(out=e0, in_=t_sb, func=AF.Exp)
                    nc.vector.tensor_tensor(
                        out=e_tiles[j], in0=e0, in1=adj_t[:, jj, :], op=OP.mult
                    )
                else:
                    e1 = work.tile([PART, N], bf16, tag="t")
                    nc.scalar.activation(
                        out=e1, in_=qb_sb, func=AF.Exp, bias=kc_sb[:, j : j + 1]
                    )
                    e2 = work.tile([PART, N], bf16, tag="e0")
                    nc.scalar.activation(
                        out=e2,
                        in_=qb_sb,
                        func=AF.Exp,
                        bias=kc2_sb[:, j : j + 1],
                        scale=LEAKY,
                    )
                    nc.vector.tensor_tensor(out=e1, in0=e1, in1=e2, op=OP.max)
                    nc.vector.tensor_tensor(
                        out=e_tiles[j], in0=e1, in1=adj_t[:, jj, :], op=OP.mult
                    )
                # denominator rows: ones[128,1].T @ e^T[j][:, chunk] accumulated
                # over j into [1, 512] psum rows; cheap gap-filler for the PE.
                for ic in range(NIC):
                    nc.tensor.matmul(
                        pden[ic],
                        lhsT=ones_col,
                        rhs=e_tiles[j][:, ic * H : (ic + 1) * H],
                        start=(j == 0),
                        stop=(j == NT - 1),
                    )

        den_row = const.tile([1, N], f32)
        for ic in range(NIC):
            nc.vector.tensor_copy(
                out=den_row[:, ic * H : (ic + 1) * H], in_=pden[ic]
            )
        nc.sync.dma_start(out=den[:], in_=den_row)

        # ---- phase C: out = e^T.T @ V (accumulate over j) ----
        out_st = None
        for t in range(NT):
            if t % GO == 0:
                out_st = opool.tile([PART, GO, H], f32, tag="ost")
            po = psC.tile([PART, H], f32, tag="po")
            # staggered j start so concurrent PSUM groups consume different
            # e-tiles while ACT is still producing them
            j0 = (4 * t) % NT
            js = [(j0 + x) % NT for x in range(NT)]
            for x, j in enumerate(js):
                nc.tensor.matmul(
                    po,
                    lhsT=e_tiles[j][:, t * PART : (t + 1) * PART],
                    rhs=V_sb[:, j, :],
                    start=(x == 0),
                    stop=(x == NT - 1),
                )
            nc.vector.tensor_copy(out=out_st[:, t % GO, :], in_=po)
            if t % GO == GO - 1:
                nc.sync.dma_start(
                    out=out[:].rearrange("(g c p) h -> g p c h", c=GO, p=PART)[
                        t // GO
                    ],
                    in_=out_st,
                )

    nc.compile()
    return nc


def get_program():
    if "nc" not in _PROGRAM_CACHE:
        _PROGRAM_CACHE["nc"] = _build_program()
    return _PROGRAM_CACHE["nc"]


def prepare_in_maps(inputs):
    feats = np.ascontiguousarray(np.asarray(inputs["feats"], dtype=np.float32))
    adj = np.asarray(inputs["adj_mat"], dtype=np.float32)
    fc_w = np.asarray(inputs["fc_w"], dtype=np.float32)
    fc_b = np.asarray(inputs["fc_b"], dtype=np.float32)
    q_w = np.asarray(inputs["q_w"], dtype=np.float32)
    q_b = np.asarray(inputs["q_b"], dtype=np.float32)
    k_w = np.asarray(inputs["k_w"], dtype=np.float32)
    k_b = np.asarray(inputs["k_b"], dtype=np.float32)

    # fold the rank-1 q/k projections through the fc layer (host, fp64)
    wq2 = fc_w.T.astype(np.float64) @ q_w[0].astype(np.float64)  # [H]
    wk2 = fc_w.T.astype(np.float64) @ k_w[0].astype(np.float64)
    bq2 = float(fc_b.astype(np.float64) @ q_w[0].astype(np.float64) + q_b[0])
    bk2 = float(fc_b.astype(np.float64) @ k_w[0].astype(np.float64) + k_b[0])

    fcwT_bf = np.ascontiguousarray(fc_w.T).astype(ml_dtypes.bfloat16)

    in_maps = []
    for b in range(BS):
        q = (feats[b].astype(np.float64) @ wq2 + bq2).astype(np.float32)  # [N]
        k = (feats[b].astype(np.float64) @ wk2 + bk2).astype(np.float32)  # [N]
        in_maps.append(
            {
                "adjT": np.ascontiguousarray(adj[b].T).astype(ml_dtypes.float8_e4m3),
                "featsT": np.ascontiguousarray(feats[b].T).astype(ml_dtypes.bfloat16),
                "fcwT": fcwT_bf,
                "qv": np.ascontiguousarray(q[None, :]).astype(ml_dtypes.bfloat16),
                "kv": np.ascontiguousarray(k.reshape(NT, PART).T),
                "kv2": np.ascontiguousarray((LEAKY * k).reshape(NT, PART).T),
            }
        )
    return in_maps, feats, fc_b


def postprocess(results, feats, fc_b):
    outs = np.empty((BS, N, H), dtype=np.float32)
    for b in range(BS):
        o = np.asarray(results[b]["out"], dtype=np.float32)  # [N, H]
        denom = np.asarray(results[b]["den"], dtype=np.float32).reshape(N)
        outs[b] = o / denom[:, None] + fc_b[None, :] + feats[b]
    return outs


def _ensure_ntff_hook():
    """This image's antenv lacks axon_hooks; shim it so trace=True works."""
    import types

    try:
        from antenv import axon_hooks  # noqa: F401

        return
    except ImportError:
        pass
    import antenv

    mod = types.ModuleType("antenv.axon_hooks")
    _hook = [None]
    mod.get_axon_ntff_profile_hook = lambda: _hook[0]
    mod.set_axon_ntff_profile_hook = lambda h: _hook.__setitem__(0, h)
    sys.modules["antenv.axon_hooks"] = mod
    antenv.axon_hooks = mod
    try:
        from trn_agent_boot.trn_boot import _ntff_profile_via_ctypes

        hook = _ntff_profile_via_ctypes("/opt/axon/libaxon_pjrt.so")
        if hook is not None:
            mod.set_axon_ntff_profile_hook(hook)
    except Exception as exc:  # degrade: run untraced
        print(f"ntff hook setup failed: {exc}", file=sys.stderr)


def run(inputs, trace=False, **kwargs):
    from concourse.bass_utils import run_bass_kernel_spmd

    if trace:
        _ensure_ntff_hook()
    in_maps, feats, fc_b = prepare_in_maps(inputs)
    nc = get_program()
    res = run_bass_kernel_spmd(
        nc, in_maps, list(range(NCORES)), trace=trace, **kwargs
    )
    return postprocess(res.results, feats, fc_b), res


def kernel(**inputs) -> np.ndarray:
    out, _ = run(inputs, trace=False)
    return out
